# Initial kernel scaffold
#
"""Trainium2 Bass kernel for a 5-layer MLP over graph nodes (ChebConv K=1 == Linear).

Network: x[50000,512] -> ELU(x@W1+b1) -> ... -> h@W5+b5, dims 512->2048(x4)->256.
The ChebConv(K=1) branch and parallel Linear branch fuse on the host into a
single weight W = Wg + Wl, bias b = bg + bl.  edge_index is unused (no
neighbor exchange for K=1).

Sharding: data-parallel over nodes, 6250 nodes per core on 8 NeuronCores,
weights replicated.  No collectives.

Per-core dataflow (all static, fully unrolled, Tile framework):
  - activations live feature-major in SBUF: H^T tiles [128 feat, Kc, NB nodes]
  - input x is loaded node-major [128,512] f32, transposed on the PE via
    identity matmuls, cast to bf16 on the DVE
  - layers 1-4: out^T[m] = sum_k W[k,m]^T @ H^T[k]   (lhsT = weight block,
    moving = activations, PSUM fp32 accumulate), ELU fused on eviction:
    elu(z) = relu(z) + exp(min(z,0)) - 1   (ACT: relu,exp; DVE: min,add,add)
  - layer 5 flips the mapping: lhsT = H^T chunk (stationary), moving = W5
    -> PSUM comes out node-major [128 nodes, 256], copied and DMA'd out.
  - weights are streamed from DRAM per 1024-node block (activations never
    leave SBUF between layers).
"""

import numpy as np
import ml_dtypes

N = 50000
IN_C = 512
DIM = 2048
OUT_C = 256
NCORES = 8
NPC = N // NCORES  # 6250 nodes per core
NB = 1024  # node block size
BLOCKS = [NB] * (NPC // NB) + ([NPC % NB] if NPC % NB else [])  # [1024]*6 + [106]
LAYER_DIMS = [(IN_C, DIM), (DIM, DIM), (DIM, DIM), (DIM, DIM), (DIM, OUT_C)]

_cache = {}


def _build():
    import concourse.bass as bass
    import concourse.tile as tile
    from concourse import bacc, mybir
    from concourse.masks import make_identity

    f32 = mybir.dt.float32
    bf16 = mybir.dt.bfloat16
    AF = mybir.ActivationFunctionType
    ALU = mybir.AluOpType

    nc = bacc.Bacc(
        trn_type="TRN2", target_bir_lowering=False, debug=False, num_devices=NCORES
    )

    x_h = nc.dram_tensor("x", [NPC, IN_C], f32, kind="ExternalInput")
    # mid-layer weights, host-packed as [16 mblk, 128 part, Kc, 128] bf16
    w_h = []
    for l, (ci, co) in enumerate(LAYER_DIMS[:4], start=1):
        kc = ci // 128
        w_h.append(
            nc.dram_tensor(f"w{l}", [co // 128, 128, kc, 128], bf16, kind="ExternalInput")
        )
    # layer-5 weight, host-packed as [128 part, 16 kblk, 256] bf16
    w5_h = nc.dram_tensor("w5", [128, DIM // 128, OUT_C], bf16, kind="ExternalInput")
    b_h = [
        nc.dram_tensor(f"b{l}", [128, DIM // 128], f32, kind="ExternalInput")
        for l in range(1, 5)
    ]
    out_h = nc.dram_tensor("out", [NPC, OUT_C], f32, kind="ExternalOutput")

    x_ap = x_h.ap()
    out_ap = out_h.ap()

    with tile.TileContext(nc) as tc:
        from contextlib import ExitStack

        with ExitStack() as ctx:
            const = ctx.enter_context(tc.tile_pool(name="const", bufs=1))
            xs = ctx.enter_context(tc.tile_pool(name="xs", bufs=3))
            a0p = ctx.enter_context(tc.tile_pool(name="a0", bufs=2))
            actp = ctx.enter_context(tc.tile_pool(name="acts", bufs=2))
            wp = ctx.enter_context(tc.tile_pool(name="wp", bufs=3))
            etp = ctx.enter_context(tc.tile_pool(name="et", bufs=2))
            outp = ctx.enter_context(tc.tile_pool(name="outp", bufs=3))
            psp = ctx.enter_context(tc.tile_pool(name="ps", bufs=5, space="PSUM"))
            tpp = ctx.enter_context(tc.tile_pool(name="tp", bufs=2, space="PSUM"))

            ident = const.tile([128, 128], f32)
            make_identity(nc, ident[:])

            w5sb = const.tile([128, DIM // 128, OUT_C], bf16)
            nc.sync.dma_start(w5sb[:], w5_h.ap())
            bsb = []
            for l in range(4):
                bt = const.tile([128, DIM // 128], f32, tag=f"b{l}")
                nc.sync.dma_start(bt[:], b_h[l].ap())
                bsb.append(bt)

            n0 = 0
            for nb in BLOCKS:
                nch = (nb + 127) // 128  # 128-node chunks in this block

                # ---- input stage: load x rows, transpose to feature-major bf16
                h0 = a0p.tile([128, IN_C // 128, NB], bf16, tag="h0")
                for c in range(nch):
                    csz = min(128, nb - c * 128)
                    xt = xs.tile([128, IN_C], f32, tag="xt")
                    nc.sync.dma_start(
                        xt[:csz, :], x_ap[n0 + c * 128 : n0 + c * 128 + csz, :]
                    )
                    for f in range(IN_C // 128):
                        pt = tpp.tile([128, 128], f32, tag="pt")
                        nc.tensor.transpose(
                            pt[:, :csz],
                            xt[:csz, f * 128 : (f + 1) * 128],
                            ident[:csz, :csz],
                        )
                        nc.vector.tensor_copy(
                            h0[:, f, c * 128 : c * 128 + csz], pt[:, :csz]
                        )

                # ---- layers 1..4 (feature-major, ELU)
                hin = h0
                for li in range(4):
                    kc = LAYER_DIMS[li][0] // 128
                    hout = actp.tile([128, DIM // 128, NB], bf16, tag="hout")
                    for m in range(DIM // 128):
                        wt = wp.tile([128, 16, 128], bf16, tag="wt")
                        nc.sync.dma_start(wt[:, :kc, :], w_h[li].ap()[m])
                        bias = bsb[li][:, m : m + 1]
                        for ch in range((nb + 511) // 512):
                            cs = min(512, nb - ch * 512)
                            sl = slice(ch * 512, ch * 512 + cs)
                            pz = psp.tile([128, 512], f32, tag="pz")
                            for k in range(kc):
                                nc.tensor.matmul(
                                    pz[:, :cs],
                                    wt[:, k, :],
                                    hin[:, k, sl],
                                    start=(k == 0),
                                    stop=(k == kc - 1),
                                )
                            # ELU eviction: relu(z+b) + exp(min(z+b,0)) - 1
                            r = etp.tile([128, 512], f32, tag="r")
                            nc.scalar.activation(
                                r[:, :cs], pz[:, :cs], AF.Relu, bias=bias, scale=1.0
                            )
                            mn = etp.tile([128, 512], f32, tag="mn")
                            nc.vector.tensor_scalar(
                                mn[:, :cs], pz[:, :cs], bias, 0.0, ALU.add, ALU.min
                            )
                            ex = etp.tile([128, 512], f32, tag="ex")
                            nc.scalar.activation(ex[:, :cs], mn[:, :cs], AF.Exp)
                            tt = etp.tile([128, 512], f32, tag="tt")
                            nc.vector.tensor_add(tt[:, :cs], r[:, :cs], ex[:, :cs])
                            nc.vector.tensor_scalar(
                                hout[:, m, sl], tt[:, :cs], -1.0, None, ALU.add
                            )
                    hin = hout

                # ---- layer 5: node-major output, no activation
                for c in range(nch):
                    csz = min(128, nb - c * 128)
                    c0 = c * 128
                    p5 = psp.tile([128, 512], f32, tag="pz")
                    for k in range(DIM // 128):
                        nc.tensor.matmul(
                            p5[:csz, :OUT_C],
                            hin[:, k, c0 : c0 + csz],
                            w5sb[:, k, :],
                            start=(k == 0),
                            stop=(k == DIM // 128 - 1),
                        )
                    ot = outp.tile([128, OUT_C], f32, tag="ot")
                    nc.scalar.copy(ot[:csz, :], p5[:csz, :OUT_C])
                    nc.sync.dma_start(
                        out_ap[n0 + c0 : n0 + c0 + csz, :], ot[:csz, :]
                    )

                n0 += nb

    nc.compile()
    return nc


def _prep_weights(inputs):
    bf16 = ml_dtypes.bfloat16
    wmaps = {}
    for l, (ci, co) in enumerate(LAYER_DIMS, start=1):
        W = np.asarray(inputs[f"Wg{l}"], np.float32) + np.asarray(
            inputs[f"Wl{l}"], np.float32
        )
        b = np.asarray(inputs[f"bg{l}"], np.float32) + np.asarray(
            inputs[f"bl{l}"], np.float32
        )
        if l < 5:
            kc = ci // 128
            wt = np.ascontiguousarray(
                W.reshape(kc, 128, co // 128, 128).transpose(2, 1, 0, 3)
            ).astype(bf16)
            wmaps[f"w{l}"] = wt
            wmaps[f"b{l}"] = np.ascontiguousarray(b.reshape(co // 128, 128).T)
        else:
            wmaps["w5"] = np.ascontiguousarray(
                W.reshape(ci // 128, 128, co).transpose(1, 0, 2)
            ).astype(bf16)
            wmaps["_b5"] = b
    return wmaps


LAST_RESULTS = None


def kernel(**inputs) -> np.ndarray:
    global LAST_RESULTS
    import os

    from concourse.bass_utils import run_bass_kernel_spmd

    x = np.asarray(inputs["x"], np.float32)
    assert x.shape == (N, IN_C)
    wmaps = _prep_weights(inputs)
    b5 = wmaps.pop("_b5")

    if "nc" not in _cache:
        _cache["nc"] = _build()
    nc = _cache["nc"]

    in_maps = []
    for c in range(NCORES):
        m = {"x": x[c * NPC : (c + 1) * NPC]}
        m.update(wmaps)
        in_maps.append(m)

    trace = bool(int(os.environ.get("KERNEL_TRACE", "0")))
    res = run_bass_kernel_spmd(
        nc, in_maps, core_ids=list(range(NCORES)), trace=trace
    )
    LAST_RESULTS = res
    out = np.concatenate([res.results[c]["out"] for c in range(NCORES)], axis=0)
    if np.any(b5):
        out = out + b5[None, :]
    return np.ascontiguousarray(out.astype(np.float32))


# revision 1
# speedup vs baseline: 1.2192x; 1.2192x over previous
"""Trainium2 Bass kernel for a 5-layer MLP over graph nodes (ChebConv K=1 == Linear).

Network: x[50000,512] -> ELU(x@W1+b1) -> ... -> h@W5+b5, dims 512->2048(x4)->256.
The ChebConv(K=1) branch and parallel Linear branch fuse on the host into a
single weight W = Wg + Wl, bias b = bg + bl.  edge_index is unused (no
neighbor exchange for K=1).

Sharding: data-parallel over nodes, 6250 nodes per core on 8 NeuronCores,
weights replicated.  No collectives.

Per-core dataflow (all static, fully unrolled, Tile framework):
  - activations live feature-major in SBUF: H^T tiles [128 feat, Kc, NB nodes]
  - input x is loaded node-major [128,512] f32, transposed on the PE via
    identity matmuls, cast to bf16 on the DVE
  - layers 1-4: out^T[m] = sum_k W[k,m]^T @ H^T[k]   (lhsT = weight block,
    moving = activations, PSUM fp32 accumulate), ELU fused on eviction:
    elu(z) = relu(z) + exp(min(z,0)) - 1   (ACT: relu,exp; DVE: min,add,add)
  - layer 5 flips the mapping: lhsT = H^T chunk (stationary), moving = W5
    -> PSUM comes out node-major [128 nodes, 256], copied and DMA'd out.
  - weights are streamed from DRAM per 1024-node block (activations never
    leave SBUF between layers).
"""

import numpy as np
import ml_dtypes

N = 50000
IN_C = 512
DIM = 2048
OUT_C = 256
NCORES = 8
NPC = N // NCORES  # 6250 nodes per core
NB = 1024  # node block size
BLOCKS = [NB] * (NPC // NB) + ([NPC % NB] if NPC % NB else [])  # [1024]*6 + [106]
LAYER_DIMS = [(IN_C, DIM), (DIM, DIM), (DIM, DIM), (DIM, DIM), (DIM, OUT_C)]

_cache = {}


def _build():
    import concourse.bass as bass
    import concourse.tile as tile
    from concourse import bacc, mybir
    from concourse.masks import make_identity

    f32 = mybir.dt.float32
    bf16 = mybir.dt.bfloat16
    AF = mybir.ActivationFunctionType
    ALU = mybir.AluOpType

    nc = bacc.Bacc(
        trn_type="TRN2", target_bir_lowering=False, debug=False, num_devices=NCORES
    )

    x_h = nc.dram_tensor("x", [NPC, IN_C], f32, kind="ExternalInput")
    # mid-layer weights, host-packed as [16 mblk, 128 part, Kc, 128] bf16
    w_h = []
    for l, (ci, co) in enumerate(LAYER_DIMS[:4], start=1):
        kc = ci // 128
        w_h.append(
            nc.dram_tensor(f"w{l}", [co // 128, 128, kc, 128], bf16, kind="ExternalInput")
        )
    # layer-5 weight, host-packed as [128 part, 16 kblk, 256] bf16
    w5_h = nc.dram_tensor("w5", [128, DIM // 128, OUT_C], bf16, kind="ExternalInput")
    b_h = [
        nc.dram_tensor(f"b{l}", [128, DIM // 128], f32, kind="ExternalInput")
        for l in range(1, 5)
    ]
    out_h = nc.dram_tensor("out", [NPC, OUT_C], f32, kind="ExternalOutput")

    x_ap = x_h.ap()
    out_ap = out_h.ap()

    with tile.TileContext(nc) as tc:
        from contextlib import ExitStack

        with ExitStack() as ctx:
            const = ctx.enter_context(tc.tile_pool(name="const", bufs=1))
            xs = ctx.enter_context(tc.tile_pool(name="xs", bufs=3))
            a0p = ctx.enter_context(tc.tile_pool(name="a0", bufs=2))
            actp = ctx.enter_context(tc.tile_pool(name="acts", bufs=2))
            wp = ctx.enter_context(tc.tile_pool(name="wp", bufs=3))
            etp = ctx.enter_context(tc.tile_pool(name="et", bufs=2))
            outp = ctx.enter_context(tc.tile_pool(name="outp", bufs=3))
            psp = ctx.enter_context(tc.tile_pool(name="ps", bufs=5, space="PSUM"))
            tpp = ctx.enter_context(tc.tile_pool(name="tp", bufs=2, space="PSUM"))

            ident = const.tile([128, 128], f32)
            make_identity(nc, ident[:])

            w5sb = const.tile([128, DIM // 128, OUT_C], bf16)
            nc.sync.dma_start(w5sb[:], w5_h.ap())
            bsb = []
            for l in range(4):
                bt = const.tile([128, DIM // 128], f32, tag=f"b{l}")
                nc.sync.dma_start(bt[:], b_h[l].ap())
                bsb.append(bt)

            n0 = 0
            for nb in BLOCKS:
                nch = (nb + 127) // 128  # 128-node chunks in this block

                # ---- input stage: load x rows, transpose to feature-major bf16
                h0 = a0p.tile([128, IN_C // 128, NB], bf16, tag="h0")
                for c in range(nch):
                    csz = min(128, nb - c * 128)
                    xt = xs.tile([128, IN_C], f32, tag="xt")
                    nc.sync.dma_start(
                        xt[:csz, :], x_ap[n0 + c * 128 : n0 + c * 128 + csz, :]
                    )
                    for f in range(IN_C // 128):
                        pt = tpp.tile([128, 128], f32, tag="pt")
                        nc.tensor.transpose(
                            pt[:, :csz],
                            xt[:csz, f * 128 : (f + 1) * 128],
                            ident[:csz, :csz],
                        )
                        nc.vector.tensor_copy(
                            h0[:, f, c * 128 : c * 128 + csz], pt[:, :csz]
                        )

                # ---- layers 1..4 (feature-major, ELU)
                hin = h0
                for li in range(4):
                    kc = LAYER_DIMS[li][0] // 128
                    hout = actp.tile([128, DIM // 128, NB], bf16, tag="hout")
                    for m in range(DIM // 128):
                        wt = wp.tile([128, 16, 128], bf16, tag="wt")
                        nc.sync.dma_start(wt[:, :kc, :], w_h[li].ap()[m])
                        bias = bsb[li][:, m : m + 1]
                        for ch in range((nb + 511) // 512):
                            cs = min(512, nb - ch * 512)
                            sl = slice(ch * 512, ch * 512 + cs)
                            pz = psp.tile([128, 512], f32, tag="pz")
                            for k in range(kc):
                                nc.tensor.matmul(
                                    pz[:, :cs],
                                    wt[:, k, :],
                                    hin[:, k, sl],
                                    start=(k == 0),
                                    stop=(k == kc - 1),
                                )
                            # ELU eviction: relu(z+b) + exp(min(z+b,0)) - 1
                            r = etp.tile([128, 512], f32, tag="r")
                            nc.scalar.activation(
                                r[:, :cs], pz[:, :cs], AF.Relu, bias=bias, scale=1.0
                            )
                            mn = etp.tile([128, 512], f32, tag="mn")
                            nc.vector.tensor_scalar(
                                mn[:, :cs], pz[:, :cs], bias, 0.0, ALU.add, ALU.min
                            )
                            ex = etp.tile([128, 512], f32, tag="ex")
                            nc.scalar.activation(ex[:, :cs], mn[:, :cs], AF.Exp)
                            tt = etp.tile([128, 512], f32, tag="tt")
                            nc.vector.tensor_add(tt[:, :cs], r[:, :cs], ex[:, :cs])
                            nc.vector.tensor_scalar(
                                hout[:, m, sl], tt[:, :cs], -1.0, None, ALU.add
                            )
                    hin = hout

                # ---- layer 5: node-major output, no activation
                for c in range(nch):
                    csz = min(128, nb - c * 128)
                    c0 = c * 128
                    p5 = psp.tile([128, 512], f32, tag="pz")
                    for k in range(DIM // 128):
                        nc.tensor.matmul(
                            p5[:csz, :OUT_C],
                            hin[:, k, c0 : c0 + csz],
                            w5sb[:, k, :],
                            start=(k == 0),
                            stop=(k == DIM // 128 - 1),
                        )
                    ot = outp.tile([128, OUT_C], f32, tag="ot")
                    nc.scalar.copy(ot[:csz, :], p5[:csz, :OUT_C])
                    nc.sync.dma_start(
                        out_ap[n0 + c0 : n0 + c0 + csz, :], ot[:csz, :]
                    )

                n0 += nb

    nc.compile()
    return nc


def _prep_weights(inputs):
    bf16 = ml_dtypes.bfloat16
    wmaps = {}
    for l, (ci, co) in enumerate(LAYER_DIMS, start=1):
        W = np.asarray(inputs[f"Wg{l}"], np.float32) + np.asarray(
            inputs[f"Wl{l}"], np.float32
        )
        b = np.asarray(inputs[f"bg{l}"], np.float32) + np.asarray(
            inputs[f"bl{l}"], np.float32
        )
        if l < 5:
            kc = ci // 128
            wt = np.ascontiguousarray(
                W.reshape(kc, 128, co // 128, 128).transpose(2, 1, 0, 3)
            ).astype(bf16)
            wmaps[f"w{l}"] = wt
            wmaps[f"b{l}"] = np.ascontiguousarray(b.reshape(co // 128, 128).T)
        else:
            wmaps["w5"] = np.ascontiguousarray(
                W.reshape(ci // 128, 128, co).transpose(1, 0, 2)
            ).astype(bf16)
            wmaps["_b5"] = b
    return wmaps


LAST_RESULTS = None


def kernel(**inputs) -> np.ndarray:
    global LAST_RESULTS
    import os

    from concourse.bass_utils import run_bass_kernel_spmd

    x = np.asarray(inputs["x"], np.float32)
    assert x.shape == (N, IN_C)
    wmaps = _prep_weights(inputs)
    b5 = wmaps.pop("_b5")

    if "nc" not in _cache:
        _cache["nc"] = _build()
    nc = _cache["nc"]

    in_maps = []
    for c in range(NCORES):
        m = {"x": x[c * NPC : (c + 1) * NPC]}
        m.update(wmaps)
        in_maps.append(m)

    trace = bool(int(os.environ.get("KERNEL_TRACE", "0")))
    res = run_bass_kernel_spmd(
        nc, in_maps, core_ids=list(range(NCORES)), trace=trace
    )
    LAST_RESULTS = res
    out = np.concatenate([res.results[c]["out"] for c in range(NCORES)], axis=0)
    if np.any(b5):
        out = out + b5[None, :]
    return np.ascontiguousarray(out.astype(np.float32))
